# revision 16
# baseline (speedup 1.0000x reference)
"""Multi-head causal attention (B=2, L=2048, D=2048, H=16) on 8 NeuronCores.

Sharding: core c = (b, g), b = c // 4 (batch), g = c % 4 (head group of 4
heads = 512 dims). Column-parallel QKV projections, local attention,
row-parallel output projection; host sums the 4 partial outputs per batch.

v4 design (vs the 293us bf16 v2): 250us per core.
- Q/K/V projections AND the output projection run in fp8-e4m3 DoubleRow
  perf mode with 3-term hi/lo error compensation:
  X@W ~ (A@C + A@D + B@C)/s where A=fp8(aX), B=fp8(aX-A), C=fp8(bW),
  D=fp8(bW-C).  24 DoubleRow matmuls replace 16 bf16 matmuls per
  (unit, head) at half the per-row cost (0.75x PE time), and the scheme
  is MORE accurate than bf16 (0.11% vs 0.24% per projection).
- hi/lo operands packed host-side into one DRAM tensor per input
  (pair-group interleaved row blocks), so DMA descriptor count and total
  bytes match the bf16 baseline.
- ctxT is evacuated as fp8 hi + lo residual during normalization
  (t = ctx/z on DVE, hi = fp8(t) on Act, lo = t - hi on DVE) feeding the
  DoubleRow output projection over head pairs.
- Attention (scores, exp, ctx) stays bf16: scores contract only dh=128
  (single k-tile) so DoubleRow cannot beat bf16 there, and fp8 exp
  weights without compensation would exceed the error budget.
- Softmax tail off the PE: gpsimd partition_all_reduce + DVE normalize;
  acc init is fused into the first add.
- PE-filler interleaving paces next-chunk Q projection and previous-chunk
  output projection inside the Act-paced attention loops; outproj psum
  groups run pair-interleaved, and the tail borrows the idle scores psum
  banks for 6 groups in flight.
"""

import numpy as np
import ml_dtypes

import concourse.bass as bass
import concourse.bacc as bacc
import concourse.mybir as mybir
import concourse.tile as tile
from concourse import bass_utils

P = 128
B, L, D, H = 2, 2048, 2048, 16
NCORES = 8
HG = NCORES // B      # 4 head groups
DG = D // HG          # 512 dims per group
HPG = DG // P         # 4 heads per group (head dim = 128)
KT = D // P           # 16 contraction tiles
NG = KT // 2          # 8 DoubleRow pair groups
T2 = 4 * NG           # 32 packed tile slices (A pairs + B pairs)
SCALE = float(1.0 / np.sqrt(D // H))
INV = 1.0 / 1024.0    # undo the 4*256 fp8 encoding scale at copy-out
f32 = mybir.dt.float32
bf16 = mybir.dt.bfloat16
fp8 = mybir.dt.float8e4
EXP = mybir.ActivationFunctionType.Exp
DR = mybir.MatmulPerfMode.DoubleRow
nbf = ml_dtypes.bfloat16
nf8 = ml_dtypes.float8_e4m3

# per-group term order: (x offset, w offset) in packed tiles
# (A,C), (A,D), (B,C)
TERMS = ((0, 0), (0, 2), (2, 0))
NDR = 3 * NG          # DoubleRow matmuls per (unit, head)


def build_nc(L_=L):
    W = min(1024, L_)     # attention q-chunk width
    NCH = L_ // W         # attention chunks
    NU = L_ // 512        # 512-wide projection column units
    WB = W // P           # 128-blocks per attention chunk

    nc = bacc.Bacc("TRN2", target_bir_lowering=False, debug=False,
                   num_devices=NCORES)
    qT = nc.dram_tensor("qT", (2 * D, L_), fp8, kind="ExternalInput").ap()
    kT = nc.dram_tensor("kT", (2 * D, L_), fp8, kind="ExternalInput").ap()
    vT = nc.dram_tensor("vT", (2 * D, L_), fp8, kind="ExternalInput").ap()
    wqT = nc.dram_tensor("wqT", (2 * D, DG), fp8, kind="ExternalInput").ap()
    wkT = nc.dram_tensor("wkT", (2 * D, DG), fp8, kind="ExternalInput").ap()
    wvT = nc.dram_tensor("wvT", (2 * D, DG), fp8, kind="ExternalInput").ap()
    woT = nc.dram_tensor("woT", (2 * DG, D), fp8, kind="ExternalInput").ap()
    tri_d = nc.dram_tensor("tri", (P, P), bf16, kind="ExternalInput").ap()
    out_d = nc.dram_tensor("out", (L_, D), bf16, kind="ExternalOutput").ap()

    from contextlib import ExitStack
    with tile.TileContext(nc) as tc:
        with ExitStack() as st:
            pool = lambda name, bufs, **kw: st.enter_context(
                tc.tile_pool(name=name, bufs=bufs, **kw))
            pers = pool("pers", 1)
            wpool = pool("wpool", 2)
            xcp = pool("xcp", 3)
            constp = pool("constp", 1)
            scps = pool("scps", 2, space="PSUM")   # [P, W] f32 (2 banks)
            cxps = pool("cxps", 1, space="PSUM")   # [P, W] f32 (2 banks)
            mmps = pool("mmps", 2, space="PSUM")   # [P, 512] f32 (1 bank)

            tri_sb = constp.tile([P, P], bf16)

            khT_sb = pers.tile([P, HPG, L_], bf16)
            vh_sb = pers.tile([P, L_ // P, DG], bf16)
            qhT_sb = pers.tile([P, HPG, L_], bf16)
            # wo packed hi/lo: index h -> hi tile of head h, 4+h -> lo
            wo_sb = pers.tile([P, 2 * HPG, D], fp8)

            # PSUM->SBUF evacuations: only Act (0) and DVE (1) may read PSUM
            def copy_out(dst, src, ei):
                if ei % 2 == 0:
                    nc.scalar.copy(dst, src)
                else:
                    nc.vector.tensor_copy(dst, src)

            # scaled evacuation for the fp8 projections (undo 1024x)
            def copy_out_scaled(dst, src, ei):
                if ei % 2 == 0:
                    nc.scalar.mul(dst, src, INV)
                else:
                    nc.vector.tensor_scalar_mul(dst, src, INV)

            # output projection evacuation (undo the 256x wo encoding)
            def copy_out_o(dst, src, ei):
                if ei % 2 == 0:
                    nc.scalar.mul(dst, src, 1.0 / 256.0)
                else:
                    nc.vector.tensor_scalar_mul(dst, src, 1.0 / 256.0)

            # ---------- projection work generators ----------
            # packed hi/lo weights as lists of (tile, g0, g1) in pair-groups
            w_sb = {}

            def load_weight(name, w_ap, halves=False):
                t = wpool.tile([P, T2, DG], fp8, tag="w", name=f"w_{name}")
                if halves:
                    for a, b in ((0, NG // 2), (NG // 2, NG)):
                        nc.sync.dma_start(
                            out=t[:, 4 * a:4 * b, :],
                            in_=w_ap[a * 512:b * 512, :].rearrange(
                                "(t p) m -> p t m", p=P))
                else:
                    nc.sync.dma_start(
                        out=t[:], in_=w_ap.rearrange("(t p) m -> p t m", p=P))
                w_sb[name] = [(t, 0, NG)]

            def wslice(name, g, off):
                """[P, 2, cols] pair slice for group g, offset 0=C, 2=D."""
                for t, a, b in w_sb[name]:
                    if a <= g < b:
                        return t[:, 4 * (g - a) + off:4 * (g - a) + off + 2, :]

            def proj_unit(name, x_ap, u, pieces=None, cp=0):
                """One 512-col unit of a q/k projection -> [dh, 512] per head.
                24 DoubleRow matmuls per head (3 compensation terms x 8
                pair-groups)."""
                src = x_ap[:, u * 512:(u + 1) * 512]
                xparts = []
                for a, b in pieces or ((0, NG),):
                    xc = xcp.tile([P, 4 * (b - a), 512], fp8, tag="xc",
                                  name=f"xc_{name}{u}_{a}")
                    nc.sync.dma_start(
                        out=xc[:],
                        in_=src[a * 512:b * 512, :].rearrange(
                            "(t p) m -> p t m", p=P))
                    xparts.append((xc, a, b))

                def xslice(g, off):
                    for t, a, b in xparts:
                        if a <= g < b:
                            return t[:, 4 * (g - a) + off:4 * (g - a) + off + 2, :]
                dst = khT_sb if name == "k" else qhT_sb
                for h in range(HPG):
                    ps = mmps.tile([P, 512], f32, tag="mm", name=f"ps_{name}")
                    idx = 0
                    for g in range(NG):
                        for xo, wo_ in TERMS:
                            yield lambda ps=ps, h=h, g=g, xo=xo, wo_=wo_, \
                                idx=idx, name=name: \
                                nc.tensor.matmul(
                                    ps[:],
                                    wslice(name, g, wo_)[:, :, h * P:(h + 1) * P],
                                    xslice(g, xo),
                                    start=(idx == 0), stop=(idx == NDR - 1),
                                    perf_mode=DR)
                            idx += 1
                    yield lambda ps=ps, dst=dst, h=h, u=u: copy_out_scaled(
                        dst[:, h, u * 512:(u + 1) * 512], ps[:], cp)

            def vproj_unit(u, split=False):
                """512 token rows of the V projection -> vh natural."""
                xc = xcp.tile([P, T2, 512], fp8, tag="xc", name=f"xc_v{u}")
                src_v = vT[:, u * 512:(u + 1) * 512]
                if split:
                    for a, b in ((0, NG // 2), (NG // 2, NG)):
                        nc.sync.dma_start(
                            out=xc[:, 4 * a:4 * b, :],
                            in_=src_v[a * 512:b * 512, :].rearrange(
                                "(t p) m -> p t m", p=P))
                else:
                    nc.sync.dma_start(
                        out=xc[:], in_=src_v.rearrange("(t p) m -> p t m", p=P))
                for lb in range(4):
                    ps = mmps.tile([P, DG], f32, tag="mm", name="ps_v")
                    idx = 0
                    for g in range(NG):
                        for xo, wo_ in TERMS:
                            yield lambda ps=ps, g=g, xo=xo, wo_=wo_, idx=idx, \
                                xc=xc, lb=lb: \
                                nc.tensor.matmul(
                                    ps[:],
                                    xc[:, 4 * g + xo:4 * g + xo + 2,
                                       lb * P:(lb + 1) * P],
                                    wslice("v", g, wo_),
                                    start=(idx == 0), stop=(idx == NDR - 1),
                                    perf_mode=DR)
                            idx += 1
                    yield lambda ps=ps, u=u, lb=lb: copy_out_scaled(
                        vh_sb[:, u * 4 + lb, :], ps[:], lb)

            def outproj_unit(C, qb, ctxT_h, split_dma=False, slots=None,
                             last_block=False):
                """One 128-q-row block of the output projection + DMA.
                fp8 DoubleRow over head pairs: 3 compensation terms x 2
                h-pairs = 6 matmuls per 512-col group.  Pairs of 512-col
                groups run interleaved so each group's evacuation overlaps
                the partner group's matmuls (psum-recycle latency hiding).
                `slots` supplies extra psum APs (tail: borrow idle scps)."""
                chi, clo = ctxT_h
                ot = otp.tile([P, D], bf16, tag="ot")
                row = out_d[(C * WB + qb) * P:(C * WB + qb + 1) * P, :]
                oterms = ((0, 0), (0, 1), (1, 0))  # (ctx hi/lo, wo hi/lo)

                def group_mms(ps, ncn):
                    idx = 0
                    for hp in range(HPG // 2):
                        for co, wo_ in oterms:
                            yield lambda ps=ps, hp=hp, co=co, wo_=wo_, \
                                qb=qb, ncn=ncn, idx=idx: \
                                nc.tensor.matmul(
                                    ps[:],
                                    (chi if co == 0 else clo)[
                                        :, 2 * hp:2 * hp + 2,
                                        qb * P:(qb + 1) * P],
                                    wo_sb[:, 4 * wo_ + 2 * hp:
                                          4 * wo_ + 2 * hp + 2,
                                          ncn * 512:(ncn + 1) * 512],
                                    start=(idx == 0),
                                    stop=(idx == 3 * HPG // 2 - 1),
                                    perf_mode=DR)
                            idx += 1

                def get_ps(ncn):
                    if slots is not None:
                        return slots(qb, ncn)
                    return mmps.tile([P, 512], f32, tag="mm", name="ps_o")[:]

                if split_dma == 2:
                    # final blocks: sequential groups; DMA [0:1024] once its
                    # two copies land, then stream the last two 512-col
                    # pieces so the closing DMA chain is short
                    for ncn in range(D // 512):
                        ps = get_ps(ncn)
                        for op in group_mms(ps, ncn):
                            yield op
                        if last_block and ncn == 3:
                            # closing chain: two 256-col copies land on Act
                            # and DVE in parallel, then a small final DMA
                            yield lambda ps=ps, ot=ot: copy_out_o(
                                ot[:, 1536:1792], ps[:, 0:256], 0)
                            yield lambda ot=ot, row=row: nc.sync.dma_start(
                                out=row[:, 1536:1792], in_=ot[:, 1536:1792])
                            yield lambda ps=ps, ot=ot: copy_out_o(
                                ot[:, 1792:2048], ps[:, 256:512], 1)
                            yield lambda ot=ot, row=row: nc.sync.dma_start(
                                out=row[:, 1792:2048], in_=ot[:, 1792:2048])
                            continue
                        yield lambda ps=ps, ot=ot, ncn=ncn: copy_out_o(
                            ot[:, ncn * 512:(ncn + 1) * 512], ps, ncn)
                        if ncn == 1:
                            yield lambda ot=ot, row=row: nc.sync.dma_start(
                                out=row[:, 0:1024], in_=ot[:, 0:1024])
                        elif ncn >= 2:
                            yield lambda ot=ot, row=row, ncn=ncn: \
                                nc.sync.dma_start(
                                    out=row[:, ncn * 512:(ncn + 1) * 512],
                                    in_=ot[:, ncn * 512:(ncn + 1) * 512])
                    return
                for nc0 in range(0, D // 512, 2):
                    psA = get_ps(nc0)
                    psB = get_ps(nc0 + 1)
                    for opA, opB in zip(group_mms(psA, nc0),
                                        group_mms(psB, nc0 + 1)):
                        yield opA
                        yield opB
                    yield lambda psA=psA, ot=ot, nc0=nc0: copy_out_o(
                        ot[:, nc0 * 512:(nc0 + 1) * 512], psA, nc0)
                    yield lambda psB=psB, ot=ot, nc0=nc0: copy_out_o(
                        ot[:, (nc0 + 1) * 512:(nc0 + 2) * 512], psB, nc0 + 1)
                if split_dma:
                    yield lambda ot=ot, row=row: nc.sync.dma_start(
                        out=row[:, 0:1024], in_=ot[:, 0:1024])
                    yield lambda ot=ot, row=row: nc.sync.dma_start(
                        out=row[:, 1024:2048], in_=ot[:, 1024:2048])
                if not split_dma:
                    yield lambda ot=ot, row=row: nc.sync.dma_start(
                        out=row, in_=ot[:])

            def kproj_pair_boot(boot):
                """K units 0+1 interleaved group-major: each weight pair is
                amortized over both units' matmuls; activations staged as
                boot pieces so the cold-start DMA supply curve stays ahead
                of the PE's piece deadlines."""
                w_sb["k"] = []
                xp01 = ([], [])
                for a, b in ((0, 1), (1, 3), (3, 5), (5, NG)):
                    wp = boot.tile([P, 4 * (b - a), DG], fp8, tag=f"bw{a}")
                    nc.sync.dma_start(
                        out=wp[:],
                        in_=wkT[a * 512:b * 512, :].rearrange(
                            "(t p) m -> p t m", p=P))
                    w_sb["k"].append((wp, a, b))
                    for u in range(2):
                        xp = boot.tile([P, 4 * (b - a), 512], fp8,
                                       tag=f"bx{a}_{u}")
                        nc.sync.dma_start(
                            out=xp[:],
                            in_=kT[a * 512:b * 512, u * 512:(u + 1) * 512]
                            .rearrange("(t p) m -> p t m", p=P))
                        xp01[u].append((xp, a, b))

                def xsl(u, g, off):
                    for t, a, b in xp01[u]:
                        if a <= g < b:
                            return t[:, 4 * (g - a) + off:4 * (g - a) + off + 2, :]
                for h in range(HPG):
                    # all four heads in flight: borrow the idle scores/ctx/mm
                    # PSUM slots so the scheduler can reorder around late
                    # activation pieces with full lookahead
                    if h < 2:
                        ps = scps.tile([P, 2, 512], f32, tag="sc",
                                       name="ps_pair")
                        halves = [ps[:, 0, :], ps[:, 1, :]]
                    elif h == 2:
                        ps = cxps.tile([P, 2, 512], f32, tag="ctx",
                                       name="ps_pair2")
                        halves = [ps[:, 0, :], ps[:, 1, :]]
                    else:
                        halves = [mmps.tile([P, 512], f32, tag="mm",
                                            name=f"ps_pair3{u}")[:]
                                  for u in range(2)]
                    idx = 0
                    for g in range(NG):
                        for xo, wo_ in TERMS:
                            for u in range(2):
                                nc.tensor.matmul(
                                    halves[u],
                                    wslice("k", g, wo_)[:, :, h * P:(h + 1) * P],
                                    xsl(u, g, xo),
                                    start=(idx == 0), stop=(idx == NDR - 1),
                                    perf_mode=DR)
                            idx += 1
                    for u in range(2):
                        copy_out_scaled(khT_sb[:, h, u * 512:(u + 1) * 512],
                                        halves[u], u)

            # ---------- emit: K proj, V proj, Q proj chunk 0 ----------
            with ExitStack() as st0:
                boot = st0.enter_context(tc.tile_pool(name="boot", bufs=1))
                if NU >= 2:
                    kproj_pair_boot(boot)
                    k_rest = range(2, NU)
                else:
                    k_rest = range(0, NU)
                    load_weight("k", wkT)
                for u in k_rest:
                    for op in proj_unit("k", kT, u):
                        op()
                load_weight("v", wvT)
            exp_p = pool("exp_p", 5)
            accp = pool("accp", 2)
            zp = pool("zp", 2)
            rzp = pool("rzp", 3)
            tp = pool("tp", 4)
            ctxtp = pool("ctxtp", 4)
            otp = pool("otp", 3)
            for u in range(NU):
                for op in vproj_unit(u, split=(u == 0)):
                    op()
                if u == min(1, NU - 1):
                    load_weight("q", wqT)
            U0 = W // 512          # q units needed for attention chunk 0
            for u in range(U0):
                for op in proj_unit("q", qT, u, cp=u):
                    op()
            nc.sync.dma_start(out=tri_sb[:], in_=tri_d)
            # wo isn't needed until the first outproj filler (chunk 1);
            # issuing it after the chunk-0 Q activations keeps it out of
            # their DMA supply chain.
            nc.sync.dma_start(
                out=wo_sb[:],
                in_=woT.rearrange("(h p) n -> p h n", p=P))

            # ---------- attention chunks with PE filler ----------
            def make_filler(C):
                def gen():
                    if C + 1 < NCH:   # next chunk's Q projection
                        for u in range((C + 1) * U0, (C + 2) * U0):
                            yield from proj_unit("q", qT, u, cp=1)
                    if C > 0:         # previous chunk's output projection
                        for qb in range(WB):
                            yield from outproj_unit(C - 1, qb,
                                                    ctxT_tiles[C - 1])
                return gen()

            # filler pacing: distribute the hidden work evenly across the
            # Act-paced inner loops; on the last chunk hold back a reserve
            # that covers the final head's normalization latency
            def fill_supply(C):
                supply = 0
                if C + 1 < NCH:
                    supply += U0 * (HPG * NDR + HPG)
                if C > 0:
                    supply += WB * (4 * 7 + 1)
                return supply

            ctxT_tiles = {}
            for C in range(NCH):
                ctxT_tiles[C] = [ctxtp.tile([P, HPG, W], fp8, tag="ctxT",
                                            name=f"ctxT{C}_{hl}")
                                 for hl in ("hi", "lo")]
                fill = make_filler(C)
                supply = fill_supply(C)
                steps = (WB * C + WB) * HPG
                reserve = 40
                paced = max(0, supply - reserve)
                state = {"credit": 0.0, "pulled": 0}

                def pump(n, force=False):
                    state["credit"] += n
                    while state["credit"] >= 1 or force:
                        if not force:
                            if state["pulled"] >= paced:
                                return
                            state["credit"] -= 1
                        op = next(fill, None)
                        if op is None:
                            return
                        op()
                        state["pulled"] += 1

                nkj = WB * C + WB
                # ctx/normalization regions finalize in cascade as the kj
                # loop passes their diagonal.  Regions MUST align to PSUM
                # banks (512 f32 cols): reading part of a bank while other
                # columns of the same bank still accumulate is unsafe.
                regions = ([(0, 512), (512, W)] if W > 512
                           else [(0, W)])

                def region_stop(C, c1):
                    return min(nkj - 1, WB * C + c1 // P - 1)
                for h in range(HPG):
                    eng = nc.vector
                    acc = accp.tile([P, W], bf16, tag="acc", name=f"acc{h}")
                    cps = cxps.tile([P, W], f32, tag="ctx", name=f"ctx{h}")
                    exs = {}

                    norm_dve = []   # DVE norm ops, drip-fed between kj steps

                    def norm_half(c0, c1):
                        # acc[:, c0:c1] and ctx psum [:, c0:c1] are final
                        # once kj passes the diagonal of column c1.
                        # Produces ctxT hi (fp8) + lo (fp8 residual) for the
                        # DoubleRow output projection: t = ctx/z in f32,
                        # hi = fp8(t) on Act, lo = fp8(t - hi) on DVE.
                        z = zp.tile([P, 512], f32, tag="z")
                        rz = rzp.tile([P, 512], f32, tag="rz")
                        chi, clo = ctxT_tiles[C]
                        nc.gpsimd.partition_all_reduce(
                            z[:, :c1 - c0], acc[:, c0:c1], channels=P,
                            reduce_op=bass.bass_isa.ReduceOp.add)
                        norm_dve.append(
                            lambda: nc.vector.reciprocal_approx_fast(
                                rz[:, :c1 - c0], z[:, :c1 - c0]))
                        last = (c0, c1) == regions[-1]
                        splits = ((c0, c1),) if last else (
                            (c0, (c0 + c1) // 2), ((c0 + c1) // 2, c1))
                        for a, b in splits:
                            t = tp.tile([P, 512], f32, tag="t")
                            norm_dve.append(
                                lambda a=a, b=b, t=t: nc.vector.tensor_mul(
                                    t[:, :b - a], cps[:, a:b],
                                    rz[:, a - c0:b - c0]))
                            norm_dve.append(
                                lambda a=a, b=b, t=t: nc.scalar.copy(
                                    chi[:, h, a:b], t[:, :b - a]))
                            norm_dve.append(
                                lambda a=a, b=b, t=t: nc.vector.tensor_sub(
                                    clo[:, h, a:b], t[:, :b - a],
                                    chi[:, h, a:b]))

                    def score_and_exp(kj):
                        j = kj - WB * C
                        joff = max(0, j) * P
                        sp = scps.tile([P, W], f32, tag="sc")
                        for c0, c1 in ((joff, 512), (max(512, joff), W)):
                            if c0 >= c1 or c1 > W:
                                continue
                            nc.tensor.matmul(
                                sp[:, c0:c1],
                                khT_sb[:, h, kj * P:(kj + 1) * P],
                                qhT_sb[:, h, C * W + c0:C * W + c1],
                                start=True, stop=True)
                        ex = exp_p.tile([P, W], bf16, tag="exp")
                        exs[kj] = (ex, joff)
                        nc.scalar.activation(
                            ex[:, joff:], sp[:, joff:], EXP, scale=SCALE)
                        if j >= 0:
                            eng.tensor_mul(
                                ex[:, joff:joff + P],
                                ex[:, joff:joff + P], tri_sb)
                        if kj == 0:
                            pass  # acc init fused into kj==1's add
                        elif kj == 1:
                            ex0 = exs[0][0]
                            if joff > 0:
                                eng.tensor_copy(acc[:, :joff], ex0[:, :joff])
                            eng.tensor_add(
                                acc[:, joff:], ex0[:, joff:], ex[:, joff:])
                        else:
                            eng.tensor_add(
                                acc[:, joff:], acc[:, joff:], ex[:, joff:])

                    def ctx_mm(kj):
                        ex, joff = exs.pop(kj)
                        for c0, c1 in regions:
                            rc0 = max(c0, joff)
                            if rc0 >= c1:
                                continue
                            nc.tensor.matmul(
                                cps[:, rc0:c1],
                                vh_sb[:, kj, h * P:(h + 1) * P],
                                ex[:, rc0:c1],
                                start=(kj == 0),
                                stop=(kj == region_stop(C, c1)))

                    for kj in range(nkj):
                        score_and_exp(kj)
                        if norm_dve:
                            norm_dve.pop(0)()
                        if kj > 0:
                            ctx_mm(kj - 1)
                            for c0, c1 in regions[:-1]:
                                if kj - 1 == region_stop(C, c1):
                                    norm_half(c0, c1)  # finalized: hide now
                        pump(paced / steps)
                    ctx_mm(nkj - 1)
                    norm_half(*regions[-1])
                    for op in norm_dve:
                        op()
                    pump(4)

                # drain any remaining filler before the next chunk
                pump(0, force=True)

            # tail: last chunk's output projection (emitted after all norm
            # muls exist -- readers must never precede their writers).
            # scores/ctx psum is idle now: borrow scps as extra outproj
            # slots so six 512-col groups pipeline against copy latency.
            scA = scps.tile([P, W], f32, tag="sc", name="otailA")
            scB = scps.tile([P, W], f32, tag="sc", name="otailB")
            cxT = cxps.tile([P, W], f32, tag="ctx", name="otailC")
            tail_slots = [scA[:, 0:512], scA[:, 512:1024],
                          scB[:, 0:512], scB[:, 512:1024],
                          cxT[:, 0:512], cxT[:, 512:1024]]

            def slots(qb, ncn):
                i = qb * 4 + ncn
                if i % 8 < 2:
                    return mmps.tile([P, 512], f32, tag="mm",
                                     name="ps_o")[:]
                return tail_slots[(i % 8) - 2]

            for qb in range(WB):
                for op in outproj_unit(NCH - 1, qb, ctxT_tiles[NCH - 1],
                                       split_dma=(2 if qb >= WB - 2 else 1),
                                       slots=slots):
                    op()
    nc.compile()
    return nc


def pack_hi_lo(xT):
    """[D', M] f32 -> [2D', M] fp8, pair-group interleaved.

    Row blocks of 256 alternate: [A(kt 2g), A(kt 2g+1), B(kt 2g), B(kt 2g+1)]
    where A = fp8(4x), B = fp8(4x - A)."""
    Dp, M = xT.shape
    A = (4.0 * xT).astype(nf8)
    Bm = (4.0 * xT - A.astype(np.float32)).astype(nf8)
    A4 = A.reshape(Dp // 256, 256, M)
    B4 = Bm.reshape(Dp // 256, 256, M)
    out = np.concatenate([A4, B4], axis=1)       # [G, 512, M]
    return np.ascontiguousarray(out.reshape(2 * Dp, M))


def make_in_maps(q, k, v, wq, wk, wv, wo):
    tri = (np.arange(P)[:, None] <= np.arange(P)[None, :]).astype(nbf)
    xT8 = {n: [pack_hi_lo(np.ascontiguousarray(x[b].T)) for b in range(len(x))]
           for n, x in (("qT", q), ("kT", k), ("vT", v))}
    # weights: scale 256, same packing; per head-group column slice
    w8 = {}
    for n, wmat in (("wqT", wq), ("wkT", wk), ("wvT", wv)):
        w8[n] = [pack_hi_lo(
            np.ascontiguousarray(wmat[g * DG:(g + 1) * DG, :].T) * 64.0)
            for g in range(HG)]
    # wo: plain hi/lo stack (rows 0..DG = hi head tiles, DG..2DG = lo)
    wo8 = []
    for g in range(HG):
        woTg = np.ascontiguousarray(wo[:, g * DG:(g + 1) * DG].T,
                                    dtype=np.float32)
        A = (256.0 * woTg).astype(nf8)
        Bm = (256.0 * woTg - A.astype(np.float32)).astype(nf8)
        wo8.append(np.ascontiguousarray(np.concatenate([A, Bm], axis=0)))
    in_maps = []
    for c in range(NCORES):
        b, g = divmod(c, HG)
        in_maps.append({
            "qT": xT8["qT"][b],
            "kT": xT8["kT"][b],
            "vT": xT8["vT"][b],
            "wqT": w8["wqT"][g],
            "wkT": w8["wkT"][g],
            "wvT": w8["wvT"][g],
            "woT": wo8[g],
            "tri": tri,
        })
    return in_maps


_nc_cache = {}


def get_nc(L_=L):
    if L_ not in _nc_cache:
        _nc_cache[L_] = build_nc(L_)
    return _nc_cache[L_]


def run(q, k, v, wq, wk, wv, wo, trace=False, L_=L):
    q, k, v, wq, wk, wv, wo = (np.asarray(x, np.float32)
                               for x in (q, k, v, wq, wk, wv, wo))
    in_maps = make_in_maps(q, k, v, wq, wk, wv, wo)
    nc = get_nc(L_)
    res = bass_utils.run_bass_kernel_spmd(
        nc, in_maps, core_ids=list(range(NCORES)), trace=trace)
    out = np.zeros((q.shape[0], L_, D), np.float32)
    for c in range(NCORES):
        b = c // HG
        out[b] += res.results[c]["out"].astype(np.float32)
    return out, res


def kernel(q, k, v, attn_mask, wq, wk, wv, wo):
    # attn_mask is the causal mask by construction; the kernel hardcodes it.
    out, _ = run(q, k, v, wq, wk, wv, wo, trace=False)
    return out


if __name__ == "__main__":
    rng = np.random.default_rng(1)
    q = rng.standard_normal((B, L, D), dtype=np.float32)
    out = kernel(q, q, q, None,
                 *(0.02 * rng.standard_normal((D, D), dtype=np.float32)
                   for _ in range(4)))
    print(out.shape, out.dtype)
